# revision 18
# baseline (speedup 1.0000x reference)
"""CapsNet dynamic-routing kernel for 8 Trainium2 NeuronCores (v2).

Sharding: tensor-parallel over N_OUT (8 output capsules per core). x_hat is
never materialized; contractions are re-expressed against W and x:

  s_t[b,o,do]     = sum_{i,di} W[o,i,do,di] * c_t[b,o,i] * x[b,i,di]
  beta_inc[b,o,i] = sum_di ( sum_do v[b,o,do] W[o,i,do,di] ) * x[b,i,di]

v2 structure per routing iteration (vs v1 baseline):
  1. Wv matmuls into a rotating 4-quarter PSUM tile; drains are [128,1024]
     ACT copies (f-scale folded) or fused DVE STT (psum*f)*x.
  2. beta increment via ONE DVE tensor_reduce over the innermost di axis
     (i-major KD layout) instead of tree-adds.
  3. beta is transposed on PE (fp32) and ACT-Exp drains the transposed PSUM
     directly -> eT[i-part, (h,b)]: no separate exp pass, no per-o e
     transposes, no Z select-matmul, no post-AR rz transposes.
  4. ZT = sum_o eT via 7 DVE adds; AllReduce runs in the transposed layout,
     CHUNKED 3x (3 ic per chunk) so rz/xz/y/s-matmuls of chunk c overlap the
     AllReduce of chunk c+1.
  5. y_o = eT_o (bcast over di) * xz, xz = xt * rzT; col-tiled s-matmuls.
  6. squash kept as [b,o]-scalar factors f (fT2), folded into drains and the
     final output multiply.
"""

import os
import sys
import types

import numpy as np
import ml_dtypes

B = 64
N_IN = 1152
D_IN = 8
N_OUT = 64
D_OUT = 16
O_LOC = 8
N_CORES = 8
KD = N_IN * D_IN  # 9216, i-major: flat index = i*8 + di
NCH = 9           # i chunks of 128
EPS = 1e-8

bf16 = ml_dtypes.bfloat16

_CACHE = {}
last_exec_ns = None

# how many of the 9 drain-pairs per g go through the fused DVE STT path
# (rest: ACT copy + DVE mul)
N_STT = 0


def _install_ntff_hook():
    try:
        import antenv
    except ImportError:
        return
    if "antenv.axon_hooks" in sys.modules:
        return
    mod = types.ModuleType("antenv.axon_hooks")
    _state = {"hook": None}
    mod.set_axon_ntff_profile_hook = lambda h: _state.__setitem__("hook", h)
    mod.get_axon_ntff_profile_hook = lambda: _state["hook"]
    sys.modules["antenv.axon_hooks"] = mod
    antenv.axon_hooks = mod
    try:
        from trn_agent_boot.trn_boot import _ntff_profile_via_ctypes
        hook = _ntff_profile_via_ctypes("/opt/axon/libaxon_pjrt.so")
        if hook is not None:
            mod.set_axon_ntff_profile_hook(hook)
    except Exception:
        pass


def _build():
    import concourse.bacc as bacc
    import concourse.tile as tile
    import concourse.mybir as mybir

    dt = mybir.dt
    Alu = mybir.AluOpType
    Act = mybir.ActivationFunctionType
    Ax = mybir.AxisListType

    nc = bacc.Bacc("TRN2", target_bir_lowering=False, debug=False,
                   num_devices=N_CORES)

    # ---- DRAM I/O ----
    d_xt = nc.dram_tensor("xt", [128, NCH, D_IN, B], dt.bfloat16,
                          kind="ExternalInput")
    d_wf = nc.dram_tensor("wf", [128, D_IN, NCH, O_LOC * D_OUT], dt.bfloat16,
                          kind="ExternalInput")
    d_wdo = nc.dram_tensor("wdo", [4, 16, 2, KD], dt.bfloat16,
                           kind="ExternalInput")
    d_xb = nc.dram_tensor("xb", [B, KD], dt.bfloat16, kind="ExternalInput")
    d_ones = nc.dram_tensor("onesbd", [128, 4], dt.float32,
                            kind="ExternalInput")
    d_idf = nc.dram_tensor("identf", [128, 128], dt.float32,
                           kind="ExternalInput")
    d_out = nc.dram_tensor("yout", [B, 2, 128], dt.float32,
                           kind="ExternalOutput")
    dbg = bool(int(os.environ.get("CAPS_DEBUG", "0")))
    if dbg:
        d_dbg_sT = nc.dram_tensor("dbg_sT", [128, 2, B], dt.bfloat16,
                                  kind="ExternalOutput")
        d_dbg_f = nc.dram_tensor("dbg_f", [128, 4], dt.float32,
                                 kind="ExternalOutput")
        d_dbg_pm = nc.dram_tensor("dbg_pm", [128, KD], dt.bfloat16,
                                  kind="ExternalOutput")
        d_dbg_beta = nc.dram_tensor("dbg_beta", [128, N_IN], dt.float32,
                                    kind="ExternalOutput")
        d_dbg_eT = nc.dram_tensor("dbg_eT", [128, NCH, 128], dt.bfloat16,
                                  kind="ExternalOutput")
        d_dbg_z = nc.dram_tensor("dbg_z", [128, NCH, B], dt.float32,
                                 kind="ExternalOutput")
        d_dbg_rz = nc.dram_tensor("dbg_rz", [128, NCH, B], dt.bfloat16,
                                  kind="ExternalOutput")
        d_dbg_xz = nc.dram_tensor("dbg_xz", [128, NCH, D_IN, B], dt.bfloat16,
                                  kind="ExternalOutput")
        d_dbg_sT1 = nc.dram_tensor("dbg_sT1", [128, 2, B], dt.bfloat16,
                                   kind="ExternalOutput")
        d_dbg_f1 = nc.dram_tensor("dbg_f1", [128, 4], dt.float32,
                                  kind="ExternalOutput")

    with tile.TileContext(nc) as tc:
        with (
            tc.tile_pool(name="const", bufs=1) as constp,
            tc.tile_pool(name="state", bufs=1) as statep,
            tc.tile_pool(name="work", bufs=1) as workp,
            tc.tile_pool(name="wk2", bufs=2) as workp2,
            tc.tile_pool(name="pswv", bufs=1, space="PSUM") as pswv,
            tc.tile_pool(name="pstr", bufs=2, space="PSUM") as pstr,
            tc.tile_pool(name="psS", bufs=1, space="PSUM") as psS,
            tc.tile_pool(name="tiny", bufs=1, space="PSUM") as tinyp,
            tc.tile_pool(name="dram", bufs=1, space="DRAM") as dramp,
        ):
            # ---------- constants / inputs ----------
            xt = constp.tile([128, NCH, D_IN, B], dt.bfloat16)
            nc.sync.dma_start(xt[:], d_xt[:])
            wf = constp.tile([128, D_IN, NCH, O_LOC * D_OUT], dt.bfloat16)
            nc.sync.dma_start(wf[:], d_wf[:])
            wdo = constp.tile([128, 2, KD], dt.bfloat16)
            for g in range(4):
                nc.sync.dma_start(wdo[32 * g:32 * g + 16, :, :], d_wdo[g])
            xb2 = constp.tile([128, KD], dt.bfloat16)
            nc.sync.dma_start(xb2[:B, :], d_xb[:])
            nc.sync.dma_start(xb2[B:, :], d_xb[:])
            onesbd = constp.tile([128, 4], dt.float32)
            nc.sync.dma_start(onesbd[:], d_ones[:])
            idf = constp.tile([128, 128], dt.float32)
            nc.sync.dma_start(idf[:], d_idf[:])

            # persistent state
            sT = statep.tile([128, 2, B], dt.bfloat16, tag="sT")
            sTf = statep.tile([128, 2, B], dt.float32, tag="sTf")
            fT2 = statep.tile([128, 4], dt.float32, tag="fT2")
            epst = statep.tile([4, 1], dt.float32, tag="epst")
            nc.gpsimd.memset(epst[:], EPS)
            beta = [statep.tile([128, N_IN], dt.float32, tag=f"beta{g}",
                                name=f"beta{g}") for g in range(4)]
            eT = [statep.tile([128, NCH, 128], dt.bfloat16, tag=f"eT{g}",
                              name=f"eT{g}") for g in range(4)]
            ZT = statep.tile([128, NCH, B], dt.float32, tag="ZT")
            zsb = statep.tile([128, NCH, B], dt.float32, tag="zsb")
            rzb = statep.tile([128, NCH, B], dt.bfloat16, tag="rzb")
            xz = statep.tile([128, NCH, D_IN, B], dt.bfloat16, tag="xz")
            zb4 = [statep.tile([128, NCH, B], dt.bfloat16, tag=f"zb{k}",
                               name=f"zb{k}") for k in range(4)]

            # beta-phase work tiles
            pbuf = workp.tile([128, KD], dt.bfloat16, tag="pbuf")
            pmul = workp.tile([128, KD], dt.bfloat16, tag="pmul")

            wv = pswv.tile([128, 4, 512], dt.float32, tag="wv")
            # one shared PSUM bank for all the small outputs
            tsh = tinyp.tile([128, 512], dt.float32, tag="tsh")

            # DRAM staging for the chunked AllReduce
            zin = [dramp.tile([128, 3, B], dt.float32, name=f"zin{c}",
                              tag=f"zin{c}") for c in range(3)]
            zout = [dramp.tile([128, 3, B], dt.float32, name=f"zout{c}",
                               tag=f"zout{c}") for c in range(3)]

            def squash(ps, first, last):
                scale = (1.0 / N_OUT) if first else 1.0
                sq = workp2.tile([128, 2, B], dt.float32, tag="sq")
                for h in range(2):
                    nc.vector.tensor_copy(sT[:, h, :], ps[:, h, :])
                    if last:
                        nc.vector.tensor_copy(sTf[:, h, :], ps[:, h, :])
                    nc.scalar.activation(sq[:, h, :], ps[:, h, :], Act.Square,
                                         scale=scale)
                n2ps = tsh[:4, :128].rearrange("p (h b) -> p h b", h=2)
                for h in range(2):
                    nc.tensor.matmul(n2ps[:, h, :], onesbd[:], sq[:, h, :],
                                     start=True, stop=True)
                n2 = workp2.tile([4, 2, B], dt.float32, tag="n2s")
                nc.vector.tensor_copy(n2[:], n2ps[:])
                srt = workp2.tile([4, 2, B], dt.float32, tag="fs")
                nc.scalar.activation(srt[:], n2[:], Act.Sqrt, bias=epst[:])
                a = workp2.tile([4, 2, B], dt.float32, tag="fa")
                # a = (n2 + 1) * sqrt(n2 + eps)
                nc.vector.scalar_tensor_tensor(a[:], n2[:], 1.0, srt[:],
                                               Alu.add, Alu.mult)
                r = workp2.tile([4, 2, B], dt.float32, tag="fr")
                nc.vector.reciprocal_approx_fast(
                    r.rearrange("g h b -> g (h b)"),
                    a.rearrange("g h b -> g (h b)"))
                f = workp2.tile([4, 2, B], dt.float32, tag="ff")
                nc.vector.tensor_mul(f[:], n2[:], r[:])
                if first:
                    nc.vector.tensor_scalar_mul(f[:], f[:], 1.0 / N_OUT)
                fps = tsh[:, 128:132]
                nc.tensor.transpose(
                    fps, f.rearrange("g h b -> g (h b)"), idf[:4, :4])
                nc.vector.tensor_copy(fT2[:], fps)

            # ---------- iteration 0 ----------
            ps0 = psS.tile([128, 2, B], dt.float32, tag="sps")
            n_acc = NCH * D_IN
            for o in range(O_LOC):
                g, h = o % 4, o // 4
                k = 0
                for ic in range(NCH):
                    for di in range(D_IN):
                        nc.tensor.matmul(
                            ps0[32 * g:32 * g + 16, h, :],
                            wf[:, di, ic, 16 * o:16 * o + 16],
                            xt[:, ic, di, :],
                            start=(k == 0), stop=(k == n_acc - 1),
                            tile_position=(0, 32 * g),
                        )
                        k += 1
            squash(ps0, first=True, last=False)
            if dbg:
                nc.sync.dma_start(d_dbg_sT[:], sT[:])
                nc.sync.dma_start(d_dbg_f[:], fT2[:])

            # ---------- iterations 1, 2 ----------
            for it in (1, 2):
                # ----- beta increment -----
                wave = 0
                for g in range(4):
                    if it == 2:
                        binc = workp2.tile([128, N_IN], dt.float32,
                                           tag="binc", name=f"binc{g}")
                    for jj in range(9):  # drain pairs (2 waves of 512 each)
                        qs = []
                        for j2 in range(2):
                            q = wave % 4
                            qs.append(q)
                            j = 2 * jj + j2
                            for h in range(2):
                                nc.tensor.matmul(
                                    wv[64 * h:64 * h + 64, q, :],
                                    sT[32 * g:32 * g + 16, h, :],
                                    wdo[32 * g:32 * g + 16, h,
                                        512 * j:512 * (j + 1)],
                                    start=True, stop=True,
                                    tile_position=(32 * g, 64 * h),
                                )
                            wave += 1
                        q0 = qs[0]
                        src = wv[:, q0:q0 + 2, :].rearrange("p a n -> p (a n)")
                        dst = pmul[:, 1024 * jj:1024 * (jj + 1)]
                        if jj < N_STT:
                            nc.vector.scalar_tensor_tensor(
                                dst, src, fT2[:, g:g + 1],
                                xb2[:, 1024 * jj:1024 * (jj + 1)],
                                Alu.mult, Alu.mult)
                        else:
                            pdst = pbuf[:, 1024 * jj:1024 * (jj + 1)]
                            nc.scalar.activation(pdst, src, Act.Copy,
                                                 scale=fT2[:, g:g + 1])
                            nc.vector.tensor_mul(
                                dst, pdst,
                                xb2[:, 1024 * jj:1024 * (jj + 1)])
                        if jj % 3 == 2:
                            rr = jj // 3
                            red = pmul[:, 3072 * rr:3072 * (rr + 1)]
                            tgt = (beta[g] if it == 1 else binc)
                            nc.vector.tensor_reduce(
                                tgt[:, 384 * rr:384 * (rr + 1)],
                                red.rearrange("p (i d) -> p i d", d=D_IN),
                                Ax.X, Alu.add)
                    if dbg and it == 1 and g == 0:
                        nc.sync.dma_start(d_dbg_pm[:], pmul[:])
                        nc.sync.dma_start(d_dbg_beta[:], beta[0][:])
                    if it == 2:
                        nc.vector.tensor_add(beta[g][:], beta[g][:], binc[:])
                    # ----- transpose beta + exp drain -> eT -----
                    for tt in range(3):
                        tr = pstr.tile([128, 3, 128], dt.float32, tag="tr",
                                       name=f"tr{it}_{g}_{tt}")
                        for u in range(3):
                            ic = 3 * tt + u
                            nc.tensor.transpose(
                                tr[:, u, :],
                                beta[g][:, 128 * ic:128 * (ic + 1)],
                                idf[:])
                        nc.scalar.activation(eT[g][:, 3 * tt:3 * tt + 3, :],
                                             tr[:], Act.Exp)

                # ----- ZT = sum_o eT, chunked AllReduce -----
                for g in range(4):
                    nc.vector.tensor_add(zb4[g][:], eT[g][:, :, :B],
                                         eT[g][:, :, B:])
                nc.vector.tensor_add(zb4[0][:], zb4[0][:], zb4[1][:])
                nc.vector.tensor_add(zb4[2][:], zb4[2][:], zb4[3][:])
                nc.vector.tensor_add(ZT[:], zb4[0][:], zb4[2][:])

                for c in range(3):
                    nc.sync.dma_start(zin[c][:], ZT[:, 3 * c:3 * c + 3, :])
                    nc.gpsimd.collective_compute(
                        "AllReduce", Alu.add,
                        ins=[zin[c].opt()], outs=[zout[c].opt()],
                        replica_groups=[list(range(N_CORES))],
                    )
                    nc.sync.dma_start(zsb[:, 3 * c:3 * c + 3, :], zout[c][:])

                if dbg and it == 1:
                    nc.sync.dma_start(d_dbg_eT[:], eT[0][:])
                    nc.sync.dma_start(d_dbg_z[:], zsb[:])

                # ----- rz, xz, y, s-matmuls (per AR chunk) -----
                psY = psS.tile([128, 2, B], dt.float32, tag="sps",
                               name=f"psY{it}")
                for c in range(3):
                    rzf = workp2.tile([128, 3, B], dt.float32, tag="rzf",
                                      name=f"rzf{it}_{c}")
                    nc.vector.reciprocal_approx_fast(
                        rzf.rearrange("p a b -> p (a b)"),
                        zsb[:, 3 * c:3 * c + 3, :]
                        .rearrange("p a b -> p (a b)"))
                    nc.vector.tensor_copy(rzb[:, 3 * c:3 * c + 3, :], rzf[:])
                    xzc = xz[:, 3 * c:3 * c + 3, :, :]
                    rbc = rzb[:, 3 * c:3 * c + 3, :].unsqueeze(2)\
                        .broadcast_to([128, 3, D_IN, B])
                    nc.vector.tensor_mul(xzc, xt[:, 3 * c:3 * c + 3, :, :],
                                         rbc)
                # o-chains kept contiguous: interleaved open accumulation
                # groups in one PSUM bank corrupt the accumulation
                for o in range(O_LOC):
                    g, h = o % 4, o // 4
                    for c in range(3):
                        yb = workp2.tile([128, 3, D_IN, B], dt.bfloat16,
                                         tag="y", name=f"y{it}_{c}_{o}")
                        ebc = eT[g][:, 3 * c:3 * c + 3, 64 * h:64 * h + 64]\
                            .unsqueeze(2).broadcast_to([128, 3, D_IN, B])
                        nc.vector.tensor_mul(yb[:], xz[:, 3 * c:3 * c + 3,
                                                       :, :], ebc)
                        for u in range(3):
                            ic = 3 * c + u
                            for di in range(D_IN):
                                k = (ic * D_IN) + di
                                nc.tensor.matmul(
                                    psY[32 * g:32 * g + 16, h, :],
                                    wf[:, di, ic, 16 * o:16 * o + 16],
                                    yb[:, u, di, :],
                                    start=(k == 0), stop=(k == n_acc - 1),
                                    tile_position=(0, 32 * g),
                                )
                squash(psY, first=False, last=(it == 2))
                if dbg and it == 1:
                    nc.sync.dma_start(d_dbg_rz[:], rzb[:])
                    nc.sync.dma_start(d_dbg_xz[:], xz[:])
                    nc.sync.dma_start(d_dbg_sT1[:], sT[:])
                    nc.sync.dma_start(d_dbg_f1[:], fT2[:])

            # ---------- final output ----------
            for h in range(2):
                op = tsh[:B, 256 + 128 * h:256 + 128 * (h + 1)]
                nc.tensor.transpose(op, sTf[:, h, :], idf[:])
                ofin = workp2.tile([B, 128], dt.float32, tag="ofin",
                                   name=f"ofin{h}")
                fbc = fT2[64 * h:64 * h + 64, :].unsqueeze(2).broadcast_to(
                    [B, 4, 32])
                nc.vector.tensor_mul(
                    ofin.rearrange("b (o r) -> b o r", o=4),
                    op.rearrange("b (o r) -> b o r", o=4),
                    fbc)
                nc.sync.dma_start(d_out[:, h, :], ofin[:])

    nc.compile()
    return nc


def _host_prep(x, W):
    xtc = np.ascontiguousarray(
        x.transpose(1, 2, 0).reshape(NCH, 128, D_IN, B)
        .transpose(1, 0, 2, 3).astype(bf16))
    xb = np.ascontiguousarray(x.reshape(B, KD).astype(bf16))
    onesbd = np.zeros((128, 4), np.float32)
    for g in range(4):
        onesbd[32 * g:32 * g + 16, g] = 1.0
    idf = np.eye(128, dtype=np.float32)

    in_maps = []
    for c in range(N_CORES):
        Wc = W[c * O_LOC:(c + 1) * O_LOC]
        wfc = np.ascontiguousarray(
            Wc.transpose(1, 3, 0, 2)
            .reshape(NCH, 128, D_IN, O_LOC * D_OUT)
            .transpose(1, 2, 0, 3).astype(bf16))
        wdoc = np.zeros((4, 16, 2, KD), np.float32)
        for g in range(4):
            for h in range(2):
                o = 4 * h + g
                # [do, (i, di)] i-major
                wdoc[g, :, h, :] = (
                    Wc[o].transpose(1, 0, 2).reshape(D_OUT, KD))
        in_maps.append({
            "xt": xtc, "wf": wfc,
            "wdo": np.ascontiguousarray(wdoc.astype(bf16)),
            "xb": xb, "onesbd": onesbd, "identf": idf,
        })
    return in_maps


def kernel(input, W):
    global last_exec_ns
    _install_ntff_hook()
    from concourse.bass_utils import run_bass_kernel_spmd

    x = np.asarray(input, dtype=np.float32)
    W = np.asarray(W, dtype=np.float32)

    if "nc" not in _CACHE:
        _CACHE["nc"] = _build()
    nc = _CACHE["nc"]

    in_maps = _host_prep(x, W)
    trace = bool(int(os.environ.get("CAPS_TRACE", "0")))
    res = run_bass_kernel_spmd(nc, in_maps, core_ids=list(range(N_CORES)),
                               trace=trace)
    last_exec_ns = res.exec_time_ns
    _CACHE["res"] = res

    outs = []
    for c in range(N_CORES):
        y = res.results[c]["yout"].reshape(B, 2, 4, 32)[:, :, :, :16]
        outs.append(y.reshape(B, 8, D_OUT))
    return np.concatenate(outs, axis=1).astype(np.float32)
